# revision 36
# baseline (speedup 1.0000x reference)
import sys
sys.path.insert(0, '/opt/trn_rl_repo')
import numpy as np
import ml_dtypes
from concurrent.futures import ThreadPoolExecutor
from contextlib import ExitStack

import concourse.bass as bass
import concourse.mybir as mybir
from concourse.bass_utils import run_bass_kernel_spmd

# Problem: y[b,s,o] = x[b]@W.T + bias + (x[b]@a[idx[b]].T)@b[idx[b]].T
# B=8 batch elements -> data-parallel, one per NeuronCore.
# The LoRA update is folded into the weight matrix on the host
# (W' = W + b[idx] @ a[idx], a rank-16 update), so the device runs a pure
# GEMM y = x @ W'.T + bias. The bias is added by the DVE during the
# PSUM->SBUF copy.
B, S, D, RANK = 8, 2048, 4096, 16
P = 128

_BF = mybir.dt.bfloat16
_F32 = mybir.dt.float32


def build_nc(s=S, d=D, warm=True):
    KT = d // P          # contraction tiles
    NQ = 4               # s-quarters
    SQ = s // NQ
    NJ = d // 512        # o-blocks: OJ=512 fills one PSUM bank exactly
    OJ = d // NJ
    NT = SQ // P         # s-tiles per quarter
    XC = 4               # x DMA chunks per quarter
    KC = KT // XC
    WC = 2               # w DMA chunks per o-block
    KW = KT // WC
    NWARM = 13           # PE warmup matmuls: bridge the DMA bootstrap window
                         # (first real mm needs xa+wa1, ready ~14.5us from
                         # NEFF start; warmup runs at ~427ns/mm while the PE
                         # p-state ramps, so it spans ~5.6us from ~8.8us)
    KH = KC // 2         # sub-chunk of the very first x chunk
    KWH = KW // 2        # sub-chunk of the very first w chunk

    nc = bass.Bass()
    xt = nc.declare_dram_parameter("xt", [d, s], _BF, isOutput=False)
    wt = nc.declare_dram_parameter("wt", [d, d], _BF, isOutput=False)
    bias_bc = nc.declare_dram_parameter("bias_bc", [P, d], _BF, isOutput=False)
    y = nc.declare_dram_parameter("y", [s, d], _BF, isOutput=True)

    xt_t = xt.rearrange("(k p) s -> p k s", p=P)
    wt_t = wt.rearrange("(k p) o -> p k o", p=P)

    with (
        nc.sbuf_tensor([P, 2, KT, SQ], _BF) as x_sb,
        nc.sbuf_tensor([P, 2, KT, OJ], _BF) as w_sb,
        nc.sbuf_tensor([P, d], _BF) as bias_sb,
        nc.sbuf_tensor([P, OJ + P], _BF) as scratch_sb,
        nc.sbuf_tensor([P, 4, OJ], _BF) as out_sb,
        nc.psum_tensor([P, 6, OJ], _F32) as psum_y,
        ExitStack() as _sems,
        nc.Block() as block,
    ):
        def _sem(name):
            return _sems.enter_context(nc.semaphore(name))

        # One semaphore per (chunk lane, double-buffer parity): at any wait
        # point only one DMA can be outstanding on a given semaphore, so
        # out-of-order per-engine completions can't satisfy a wait early.
        x_sems = [[_sem(f"x_sem{c}_{p}") for p in range(2)] for c in range(XC)]
        w_sems = [[_sem(f"w_sem{h}_{p}") for p in range(2)] for h in range(WC)]
        c_sem = _sem("c_sem")
        pe_sem = _sem("pe_sem")
        ev_sem = _sem("ev_sem")
        st_sems = [_sem(f"st_sem{p}") for p in range(4)]
        fin_sem = _sem("fin_sem")
        # dedicated sems for the extra-fine first x/w sub-chunks (quarter 0,
        # j-block 0 only) — each has exactly one DMA, so waits are race-free
        xa_sem = _sem("xa_sem")
        xb_sem = _sem("xb_sem")
        wa1_sem = _sem("wa1_sem")
        wa2_sem = _sem("wa2_sem")
        wb_sem = _sem("wb_sem")
        bw_sem = _sem("bw_sem")    # bias pre-written into the last psum bank
        xc1a_sem = _sem("xc1a_sem")
        xc1b_sem = _sem("xc1b_sem")
        xc2a_sem = _sem("xc2a_sem")
        xc2b_sem = _sem("xc2b_sem")
        xc3a_sem = _sem("xc3a_sem")
        xc3b_sem = _sem("xc3b_sem")
        w1a_sem = _sem("w1a_sem")
        w1b_sem = _sem("w1b_sem")

        def _dma_x(eng, q):
            for c in range(XC):
                eng.dma_start(
                    x_sb[:, q % 2, c * KC:(c + 1) * KC, :],
                    xt_t[:, c * KC:(c + 1) * KC, q * SQ:(q + 1) * SQ],
                ).then_inc(x_sems[c][q % 2], 16)

        def _dma_w(eng, wj, h):
            j = wj % NJ
            eng.dma_start(
                w_sb[:, j % 2, h * KW:(h + 1) * KW, :],
                wt_t[:, h * KW:(h + 1) * KW, j * OJ:(j + 1) * OJ],
            ).then_inc(w_sems[h][j % 2], 16)

        def _dma_w_full(eng, wj):
            # whole j-block in one DMA: fewer DMA starts -> fewer stolen
            # PE slots (each DMA start costs ~one 216ns matmul slot)
            j = wj % NJ
            eng.dma_start(
                w_sb[:, j % 2, :, :],
                wt_t[:, :, j * OJ:(j + 1) * OJ],
            ).then_inc(w_sems[0][j % 2], 16)

        def _dma_xh(sync, k0, k1, sem):
            sync.dma_start(
                x_sb[:, 0, k0:k1, :], xt_t[:, k0:k1, 0:SQ]
            ).then_inc(sem, 16)

        @block.scalar
        def _(scalar):
            # j-block-0 w chunks ride the scalar engine's DMA queue, in
            # parallel with the x chunks on the sync queue: the two queues'
            # ring-init latencies overlap, so the first matmul's inputs
            # (xa + wa1) are ready ~4us earlier.
            scalar.dma_start(
                w_sb[:, 0, 0:KH, :], wt_t[:, 0:KH, 0:OJ]
            ).then_inc(wa1_sem, 16)
            scalar.dma_start(
                w_sb[:, 0, KH:KWH, :], wt_t[:, KH:KWH, 0:OJ]
            ).then_inc(wa2_sem, 16)
            scalar.dma_start(
                w_sb[:, 0, KWH:KW, :], wt_t[:, KWH:KW, 0:OJ]
            ).then_inc(wb_sem, 16)
            scalar.dma_start(
                w_sb[:, 0, KW:KW + KWH, :], wt_t[:, KW:KW + KWH, 0:OJ]
            ).then_inc(w1a_sem, 16)
            scalar.dma_start(
                w_sb[:, 0, KW + KWH:KT, :], wt_t[:, KW + KWH:KT, 0:OJ]
            ).then_inc(w1b_sem, 16)


        @block.sync
        def _(sync):
            # x chunks of quarter 0, split extra fine so the very first
            # matmuls start as early as possible.
            _dma_xh(sync, 0, KH, xa_sem)
            _dma_xh(sync, KH, KC, xb_sem)
            _dma_xh(sync, KC, KC + KH, xc1a_sem)
            _dma_xh(sync, KC + KH, 2 * KC, xc1b_sem)
            _dma_xh(sync, KW, KW + KH, xc2a_sem)
            _dma_xh(sync, KW + KH, KW + KC, xc2b_sem)
            _dma_xh(sync, KW + KC, KW + KC + KH, xc3a_sem)
            _dma_xh(sync, KW + KC + KH, KT, xc3b_sem)
            # both w j1 chunks before the bias constant: j1's matmuls need
            # them ~30us earlier than the first bias-add needs bias
            _dma_w(sync, 1, 0)
            _dma_w(sync, 1, 1)
            sync.dma_start(bias_sb[:, :], bias_bc[:, :]).then_inc(c_sem, 16)
            for q in range(NQ):
                if q >= 2:
                    sync.wait_ge(ev_sem, NJ * NT * (q - 1))
                if q > 0:
                    _dma_x(sync, q)
                for j in range(2 if q == 0 else 0, NJ):
                    wj = q * NJ + j
                    if wj >= 2:
                        sync.wait_ge(ev_sem, NT * (wj - 1))
                    _dma_w_full(sync, wj)
            # Final stores go through this idle HW-DGE queue: the gpsimd
            # SW-DGE ring then drains ~3us earlier, shortening the tail.
            # (fin_sem is HW-DGE-only; nothing waits on it — the epilogue
            # DRAIN guarantees completion.)
            NG = NQ * NJ * NT
            for g in range(NG - 4, NG - 1):
                q, rem = divmod(g, NJ * NT)
                j, t = divmod(rem, NT)
                st = q * NT + t
                sync.wait_ge(ev_sem, g + 1)
                sync.dma_start(
                    y[st * P:(st + 1) * P, j * OJ:(j + 1) * OJ], out_sb[:, g % 4, :]
                ).then_inc(fin_sem, 16)
            # last group streams out as two 256-col halves; h0 goes out on
            # the gpsimd ring in parallel so the two ~0.6us descriptor
            # generations don't serialize on this queue
            g = NG - 1
            q, rem = divmod(g, NJ * NT)
            j, t = divmod(rem, NT)
            st = q * NT + t
            sync.wait_ge(ev_sem, g + 2)
            sync.dma_start(
                y[st * P:(st + 1) * P, j * OJ + 256:(j + 1) * OJ],
                out_sb[:, g % 4, 256:512],
            ).then_inc(fin_sem, 16)

        @block.tensor
        def _(tensor):
            # Warm the PE (HAM un-throttle) on scratch data while the first
            # input DMAs are still in flight.
            for _ in range(NWARM if warm else 0):
                nc.tensor.matmul(
                    psum_y[:, 0, :], scratch_sb[:, OJ:OJ + P], scratch_sb[:, 0:OJ],
                    start=True, stop=True,
                )

            _xq0 = [(xa_sem, xb_sem), (xc1a_sem, xc1b_sem),
                    (xc2a_sem, xc2b_sem), (xc3a_sem, xc3b_sem)]

            def _x_wait(q, c):
                if q == 0:
                    tensor.wait_ge(_xq0[c][0], 16)
                    tensor.wait_ge(_xq0[c][1], 16)
                    return
                # quarter 0 is entirely off the x_sems lanes
                tensor.wait_ge(x_sems[c][q % 2], 16 * ((q + 1) // 2))

            def _w_wait(wj, h):
                # wj=1 ships as two chunks on lanes [0][1]/[1][1]; wj>=2 is a
                # single full-block DMA on lane [0][parity]. Lane [0][1] incs:
                # wj=1,3,5,..; lane [0][0]: wj=2,4,.. (block 0 is off-lane) ->
                # value after block wj's DMA = 16*((wj+1)//2) on both lanes.
                if wj == 1:
                    tensor.wait_ge(w_sems[h][1], 16)
                    return
                assert h == 0
                tensor.wait_ge(w_sems[0][wj % 2], 16 * ((wj + 1) // 2))

            # Quarter 0, j-block 0: run the matmuls paced chunk by chunk as
            # x/w interleave on the DMA queue — real work starts ~10us
            # earlier than waiting for the full quarter.
            _ranges = [
                (0, KH, [(xa_sem, 16), (wa1_sem, 16)]),
                (KH, KC, [(xb_sem, 16), (wa2_sem, 16)]),
                (KC, KC + KH, [(xc1a_sem, 16), (wb_sem, 16)]),
                (KC + KH, KW, [(xc1b_sem, 16)]),
                (KW, KW + KH, [(xc2a_sem, 16), (w1a_sem, 16)]),
                (KW + KH, KW + KC, [(xc2b_sem, 16)]),
                (KW + KC, KW + KC + KH, [(xc3a_sem, 16), (w1b_sem, 16)]),
                (KW + KC + KH, KT, [(xc3b_sem, 16)]),
            ]
            for lo, hi, waits in _ranges:
                for sem, th in waits:
                    tensor.wait_ge(sem, th)
                for t in range(NT):
                    for i in range(lo, hi):
                        mm = nc.tensor.matmul(
                            psum_y[:, t, :],
                            x_sb[:, 0, i, t * P:(t + 1) * P],
                            w_sb[:, 0, i, :],
                            start=(i == 0), stop=(i == KT - 1),
                        )
                        if i == KT - 1:
                            mm.then_inc(pe_sem, 1)

            g = NT
            for q in range(NQ):
                for j in range(1 if q == 0 else 0, NJ):
                    wj = q * NJ + j
                    _w_wait(wj, 0)
                    need_h2 = True
                    for t in range(NT):
                        if g >= 6:
                            tensor.wait_ge(ev_sem, g - 5)
                        # last group: the DVE pre-writes bias into this bank,
                        # so accumulate on top (start=False) — the tail then
                        # only needs fast scalar copies instead of DVE adds
                        last_g = g == NQ * NJ * NT - 1
                        if last_g:
                            tensor.wait_ge(bw_sem, 1)
                        for i in range(KW):
                            if q > 0 and j == 0 and t == 0 and i % KC == 0:
                                _x_wait(q, i // KC)
                            nc.tensor.matmul(
                                psum_y[:, g % 6, :],
                                x_sb[:, q % 2, i, t * P:(t + 1) * P],
                                w_sb[:, j % 2, i, :],
                                start=(i == 0 and not last_g), stop=False,
                                skip_group_check=last_g,
                            )
                        if need_h2:
                            if wj == 1:
                                _w_wait(wj, 1)
                            need_h2 = False
                        for i in range(KW, KT):
                            if q > 0 and j == 0 and t == 0 and i % KC == 0:
                                _x_wait(q, i // KC)
                            mm = nc.tensor.matmul(
                                psum_y[:, g % 6, :],
                                x_sb[:, q % 2, i, t * P:(t + 1) * P],
                                w_sb[:, j % 2, i, :],
                                start=False, stop=(i == KT - 1),
                                skip_group_check=last_g,
                            )
                        mm.then_inc(pe_sem, 1)
                        g += 1

        @block.vector
        def _(vector):
            NG = NQ * NJ * NT
            vector.wait_ge(c_sem, 16)  # bias broadcast loaded
            for g in range(NG - 1):
                q, rem = divmod(g, NJ * NT)
                j, t = divmod(rem, NT)
                vector.wait_ge(pe_sem, g + 1)
                if g >= 4:
                    vector.wait_ge(st_sems[g % 4], 16 * (g // 4))
                nc.vector.tensor_add(
                    out_sb[:, g % 4, :], psum_y[:, g % 6, :],
                    bias_sb[:, j * OJ:(j + 1) * OJ],
                ).then_inc(ev_sem, 1)
                if g == NG - 7:
                    # bank (NG-1)%6 just freed by this add: pre-write bias so
                    # the last group accumulates on top of it
                    nc.vector.tensor_copy(
                        psum_y[:, (NG - 1) % 6, :],
                        bias_sb[:, (NJ - 1) * OJ:NJ * OJ],
                    ).then_inc(bw_sem, 1)
            # tail: the last group's psum already contains bias, so two fast
            # half-copies finish the output — the h0 store overlaps the h1
            # copy (no bias-add needed on the critical tail path)
            vector.wait_ge(pe_sem, NG)
            vector.wait_ge(st_sems[(NG - 1) % 4], 16 * ((NG - 1) // 4))
            for h in range(2):
                nc.vector.tensor_copy(
                    out_sb[:, (NG - 1) % 4, h * 256:(h + 1) * 256],
                    psum_y[:, (NG - 1) % 6, h * 256:(h + 1) * 256],
                ).then_inc(ev_sem, 1)

        @block.gpsimd
        def _(gpsimd):
            NG = NQ * NJ * NT
            for g in range(NG - 4):
                q, rem = divmod(g, NJ * NT)
                j, t = divmod(rem, NT)
                st = q * NT + t
                gpsimd.wait_ge(ev_sem, g + 1)
                gpsimd.dma_start(
                    y[st * P:(st + 1) * P, j * OJ:(j + 1) * OJ], out_sb[:, g % 4, :]
                ).then_inc(st_sems[g % 4], 16)
            # last group's h0 half (sync does h1 concurrently)
            g = NG - 1
            q, rem = divmod(g, NJ * NT)
            j, t = divmod(rem, NT)
            st = q * NT + t
            gpsimd.wait_ge(ev_sem, g + 1)
            gpsimd.dma_start(
                y[st * P:(st + 1) * P, j * OJ:j * OJ + 256],
                out_sb[:, g % 4, 0:256],
            ).then_inc(st_sems[g % 4], 16)

    return nc


_NC_CACHE = {}


def _get_nc():
    if "nc" not in _NC_CACHE:
        _NC_CACHE["nc"] = build_nc()
    return _NC_CACHE["nc"]


def _conv_x(xc):
    return np.ascontiguousarray(xc.astype(np.float32).T).astype(ml_dtypes.bfloat16)


def _conv_w(args):
    W, a, b = args
    # wt[k, o] = (W + b @ a).T = W.T + a.T @ b.T
    wt = W.astype(np.float32).T + a.astype(np.float32).T @ b.astype(np.float32).T
    return np.ascontiguousarray(wt).astype(ml_dtypes.bfloat16)


def make_in_maps(x, W, bias, lora_a, lora_b, adapter_indices):
    bias_bc = np.broadcast_to(
        bias.astype(np.float32).astype(ml_dtypes.bfloat16), (P, D)
    ).copy()
    with ThreadPoolExecutor(max_workers=2 * B) as ex:
        xts = ex.map(_conv_x, [x[c] for c in range(B)])
        wts = ex.map(_conv_w, [
            (W, lora_a[int(adapter_indices[c])], lora_b[int(adapter_indices[c])])
            for c in range(B)
        ])
        xts, wts = list(xts), list(wts)
    return [{"xt": xts[c], "wt": wts[c], "bias_bc": bias_bc} for c in range(B)]


def kernel(x, W, bias, lora_a, lora_b, adapter_indices):
    nc = _get_nc()
    in_maps = make_in_maps(x, W, bias, lora_a, lora_b, adapter_indices)
    res = run_bass_kernel_spmd(nc, in_maps, list(range(B)))
    out = np.stack([res.results[c]["y"] for c in range(B)], axis=0)
    return out.astype(np.float32)
